# revision 1
# baseline (speedup 1.0000x reference)
"""Trainium2 Bass kernel: Mistral-style GQA attention with sliding-window mask.

Problem: hidden [1,2048,4096] -> Wq/Wk/Wv projections (32 q heads, 8 kv heads,
head_dim 128) -> RoPE -> sliding-window (1024) causal attention -> Wo.

Sharding: tensor-parallel over heads on 8 NeuronCores. Core i owns KV head i
and query heads 4i..4i+3 (Wq/Wk/Wv row-sharded, Wo column-sharded). Each core
computes partial_i = attn_heads_i @ Wo_i^T in HBM; host sums the 8 partials
(the TP all-reduce) to produce the full output.

All inputs are pre-cast to bf16 and pre-transposed on the host; each core
writes a bf16 [2048, 4096] partial that the host accumulates in fp32.

On-device per core, per 512-query chunk (pipelined):
  proj: stream H^T bf16 tiles from HBM in two 256-col seq halves, matmul
        into Q^T/K^T/V^T [head_dim, seq] (6 PSUM banks), RoPE on DVE/GP out
        of PSUM under the second half's k-loop, V^T -> natural [seq, head_dim]
        via DMA-transpose. The previous chunk's 32 Wo output tiles (4 head-
        matmuls each, 2 PSUM banks) interleave into the k-loop and the
        attention pre-loop, riding across chunk AND iteration boundaries.
  attn: block-sparse sliding-window attention: scores^T = K^T-block x
        Q^T-chunk on PE (start-anywhere PSUM accumulation over exact visible
        slices), exp on ACT (PSUM -> SBUF bf16), triangular edge masks on
        DVE, P@V + ones-vector row-sum denominators accumulated on PE,
        P@V staged out via ACT so the banks recycle, then
        reciprocal+partition_broadcast normalization into bf16 tiles that
        feed the next chunk's Wo matmuls.
"""

import sys

for _p in ("/opt/trn_rl_repo", "/root/.axon_site/_ro/trn_rl_repo"):
    if _p not in sys.path:
        sys.path.insert(0, _p)

import numpy as np
import ml_dtypes

import concourse.bass as bass  # noqa: F401  (registers engine classes)
import concourse.mybir as mybir
import concourse.tile as tile
from concourse import bacc
from concourse.bass_utils import run_bass_kernel_spmd

S = 2048
HID = 4096
D = 128
NQH = 4          # query heads per core
NCORES = 8
SC = 512         # seq chunk
NCH = S // SC
KT = HID // 128  # contraction tiles
WINDOW = 1024
ROPE_BASE = 10000.0
SCALE = 1.0 / float(np.sqrt(D))

F32 = mybir.dt.float32
BF16 = mybir.dt.bfloat16
MULT = mybir.AluOpType.mult
ADD = mybir.AluOpType.add
SUB = mybir.AluOpType.subtract
EXP = mybir.ActivationFunctionType.Exp

# ptb slot layout: slot sl = kb - 4c + 8 for key-block kb in chunk c.
# exp-written region per slot, and statically-zero (memset once) regions.
def _slot_region(sl):
    lo = 128 * (sl - 8) if sl >= 8 else 0
    hi = 512 if sl >= 3 else 128 * (sl + 1)
    return lo, hi


def _program(tc, dr, out, niter=1, fused=True):
    nc = tc.nc
    ht, wqt, wkt, wvt, wot = dr["ht"], dr["wqt"], dr["wkt"], dr["wvt"], dr["wot"]
    ctab, stab, mcaus, mwin = dr["ctab"], dr["stab"], dr["mcaus"], dr["mwin"]

    def _copy(eng, out_ap, in_ap):
        if eng is nc.scalar:
            eng.copy(out_ap, in_ap)
        else:
            eng.tensor_copy(out_ap, in_ap)

    # Wo PSUM->SBUF copies: DVE mid-chunk (it is idle there), ACT for the
    # boundary groups (DVE ropes then; a Wo copy queued behind rope ops
    # holds its PSUM bank and stalls PE's next Wo group)
    def pcopy_rr(out_ap, in_ap, eng=None):
        _copy(eng or nc.scalar, out_ap, in_ap)

    from contextlib import ExitStack
    if True:
        with ExitStack() as ctx:
            pw = ctx.enter_context(tc.tile_pool(name="persist", bufs=1))
            prt = ctx.enter_context(tc.tile_pool(name="ropet", bufs=2))

            wqb = pw.tile([128, KT * 512], BF16, name="wqb")
            wkb = pw.tile([128, KT * 128], BF16, name="wkb")
            wvb = pw.tile([128, KT * 128], BF16, name="wvb")
            wob = pw.tile([128, NQH * HID], BF16, name="wob")
            qtb = [pw.tile([128, S], BF16, name=f"qtb{h}") for h in range(NQH)]
            ktb = pw.tile([128, S], BF16, name="ktb")
            vtb = pw.tile([128, S], BF16, name="vtb")
            vnat = pw.tile([128, S], BF16, name="vnat")
            cs_t = pw.tile([128, S], F32, name="cs_t")
            sn_t = pw.tile([128, S], F32, name="sn_t")
            mc_t = pw.tile([128, 128], BF16, name="mc_t")
            mw_t = pw.tile([128, 128], BF16, name="mw_t")
            ones_t = pw.tile([128, 1], BF16, name="ones_t")

            nc.gpsimd.memset(ones_t[:], 1.0)

            def _rope(dst, p, c, co, w, eng="dve"):
                """dst[bf16 [128,w] slice] = rope(p [[128,w] slice, PSUM or SBUF]),
                chunk c col-offset co.

                cs_t is cos duplicated across both halves; sn_t is sign-baked
                sin: rows 0:64 = -sin, rows 64:128 = +sin, so
                out = q*cos + rot(q)*sn with rot a plain half-swap.
                """
                use_gp = eng == "gp"
                lo, hi = SC * c + co, SC * c + co + w
                csl = cs_t[:, lo:hi]
                snl = sn_t[:, lo:hi]
                if use_gp:
                    pre = prt.tile([128, w], BF16, tag="rpre", name="rpre", bufs=2)
                    rot = prt.tile([128, w], BF16, tag="rrot", name="rrot", bufs=2)
                    nc.scalar.copy(pre[:], p[:])
                    nc.scalar.copy(rot[0:64, :], p[64:128, :])
                    nc.scalar.copy(rot[64:128, :], p[0:64, :])
                    g1 = prt.tile([128, w], F32, tag="rt1", name="g1")
                    g2 = prt.tile([128, w], F32, tag="rt2", name="g2")
                    nc.gpsimd.tensor_tensor(g1[:], pre[:], csl, MULT)
                    nc.gpsimd.tensor_tensor(g2[:], rot[:], snl, MULT)
                    nc.gpsimd.tensor_tensor(dst[:, :], g1[:], g2[:], ADD)
                    return
                e = nc.vector
                t1 = prt.tile([64, w], F32, tag="rt1", name="rt1")
                t2 = prt.tile([64, w], F32, tag="rt2", name="rt2")
                e.tensor_tensor(t1[:], p[0:64, :], csl[0:64, :], MULT)
                e.tensor_tensor(t2[:], p[64:128, :], snl[0:64, :], MULT)
                e.tensor_tensor(dst[0:64, :], t1[:], t2[:], ADD)
                t3 = prt.tile([64, w], F32, tag="rt1", name="rt3")
                t4 = prt.tile([64, w], F32, tag="rt2", name="rt4")
                e.tensor_tensor(t3[:], p[64:128, :], csl[64:128, :], MULT)
                e.tensor_tensor(t4[:], p[0:64, :], snl[64:128, :], MULT)
                e.tensor_tensor(dst[64:128, :], t3[:], t4[:], ADD)

            phb = ctx.enter_context(tc.tile_pool(name="htbp", bufs=12))
            ppt = ctx.enter_context(tc.tile_pool(name="ptp", bufs=8))
            pmisc = ctx.enter_context(tc.tile_pool(name="miscb", bufs=2))
            pat = ctx.enter_context(tc.tile_pool(name="atbp", bufs=8))
            posb = ctx.enter_context(tc.tile_pool(name="osbp", bufs=2))
            # Wo-output PSUM pool stays open across proj+attn of every chunk:
            # 2 banks here + 6 proj banks = 8; 2 + 6 attn banks = 8.
            pop = ctx.enter_context(tc.tile_pool(name="outps", bufs=2, space="PSUM"))

            # Wo emission: one group = one [128q, 512hid] output tile of the
            # PREVIOUS chunk; groups interleave into the proj k-loop (PSUM
            # budget) and the attn pre-loop (covers the rope-tail window).
            wo_state = {"c": None, "atbs": None, "m": 0, "obig": None}

            def emit_wo_group():
                st = wo_state
                if st["c"] is None or st["m"] >= 32:
                    return False
                wj, wn = st["m"] // 8, st["m"] % 8
                wc, watbs = st["c"], st["atbs"]
                if wn % 4 == 0:
                    st["obig"] = posb.tile([128, HID // 2], BF16, tag="osb", name="osb")
                po = pop.tile([128, 512], F32, tag="po", name="po")
                for h in range(NQH):
                    nc.tensor.matmul(po[:], watbs[h][:, 128 * wj:128 * (wj + 1)],
                                     wob[:, HID * h + 512 * wn:HID * h + 512 * (wn + 1)],
                                     start=(h == 0), stop=(h == NQH - 1))
                pcopy_rr(st["obig"][:, 512 * (wn % 4):512 * (wn % 4 + 1)], po[:],
                         eng=(nc.vector if st["m"] < 24 else nc.scalar))
                if wn % 4 == 3:
                    nc.scalar.dma_start(
                        out[SC * wc + 128 * wj:SC * wc + 128 * (wj + 1),
                            2048 * (wn // 4):2048 * (wn // 4 + 1)],
                        st["obig"][:])
                st["m"] += 1
                return True

            def proj_stage(c):
                # projections for chunk c, in two 256-col seq halves: rope of
                # half 0 runs on DVE/GP/ACT underneath PE's half-1 k-loop, so
                # only the half-1 rope tail is exposed at the chunk boundary
                # (the attn PSUM pools can only open once the proj pool closes,
                # i.e. after the last rope drains its bank). Chunk 0 stays
                # full-width — its k-loop must cover the serial weight-DMA
                # stream — and stages q2/q3 through SBUF so the pool close
                # only waits on the K/q0/q1 ropes.
                halves = 2
                w = SC // halves
                with tc.tile_pool(name="projps", bufs=6, space="PSUM") as ppp:
                    ps6 = [ppp.tile([128, 512], F32, tag="proj", name=f"proj{c}_{i}")
                           for i in range(6)]
                    it = 0
                    for half in range(halves):
                        co = w * half
                        hb = None
                        for k in range(KT):
                            g = k // 4
                            if k % 4 == 0:
                                hb = phb.tile([128, 4 * w], BF16,
                                              tag="htb", name="hb", bufs=12)
                                nc.sync.dma_start(
                                    hb[:].rearrange("p (k j) -> p k j", j=w),
                                    ht[512 * g:512 * (g + 1),
                                       SC * c + co:SC * c + co + w].rearrange(
                                        "(k p) j -> p k j", p=128))
                                if c == 0 and g < 4:
                                    nc.sync.dma_start(
                                        wqb[:, 4096 * g:4096 * (g + 1)].rearrange(
                                            "p (k j) -> p k j", j=512),
                                        wqt[1024 * g:1024 * (g + 1), :].rearrange(
                                            "(k p) j -> p k j", p=128))
                                    nc.sync.dma_start(
                                        wkb[:, 1024 * g:1024 * (g + 1)].rearrange(
                                            "p (k j) -> p k j", j=128),
                                        wkt[1024 * g:1024 * (g + 1), :].rearrange(
                                            "(k p) j -> p k j", p=128))
                                    nc.sync.dma_start(
                                        wvb[:, 1024 * g:1024 * (g + 1)].rearrange(
                                            "p (k j) -> p k j", j=128),
                                        wvt[1024 * g:1024 * (g + 1), :].rearrange(
                                            "(k p) j -> p k j", p=128))
                                if c == 0 and g == 2:
                                    nc.sync.dma_start(cs_t[:, 0:SC], ctab[:, 0:SC])
                                    nc.sync.dma_start(sn_t[:, 0:SC], stab[:, 0:SC])
                                    nc.sync.dma_start(mc_t[:], mcaus[:])
                                    nc.sync.dma_start(mw_t[:], mwin[:])
                                if c == 0 and half == 1 and g < 4:
                                    # wob halves ride c0's second half (only
                                    # h-tiles stream there; half 0 carries
                                    # the whole weight stream)
                                    wh = 2 * g
                                    for j in (wh, wh + 1):
                                        nc.sync.dma_start(
                                            wob[:, 2048 * j:2048 * (j + 1)],
                                            wot[128 * (j // 2):128 * (j // 2 + 1),
                                                4096 * (j % 2) // 2:
                                                4096 * (j % 2) // 2 + 2048])
                                if c == 1 and half == 0 and g == 0:
                                    nc.sync.dma_start(cs_t[:, SC:], ctab[:, SC:])
                                    nc.sync.dma_start(sn_t[:, SC:], stab[:, SC:])
                            hsl = hb[:, w * (k % 4):w * (k % 4 + 1)]
                            first, last = k == 0, k == KT - 1
                            for h in range(NQH):
                                nc.tensor.matmul(
                                    ps6[h][:, co:co + w],
                                    wqb[:, 512 * k + 128 * h:512 * k + 128 * (h + 1)],
                                    hsl, start=first, stop=last, skip_group_check=True)
                            nc.tensor.matmul(ps6[4][:, co:co + w],
                                             wkb[:, 128 * k:128 * (k + 1)], hsl,
                                             start=first, stop=last, skip_group_check=True)
                            nc.tensor.matmul(ps6[5][:, co:co + w],
                                             wvb[:, 128 * k:128 * (k + 1)], hsl,
                                             start=first, stop=last, skip_group_check=True)
                            if it >= 8 and wo_state["m"] < 24:
                                emit_wo_group()
                            it += 1
                        if half == halves - 1:
                            # their ACT copies precede the rope pre-copies in
                            # ACT's in-order queue, keeping the po-bank ring
                            # turning while DVE ropes
                            emit_wo_group()
                            emit_wo_group()
                        for r0 in range(0, w, 256):
                            # q1 before K on DVE: the attn score pool reuses
                            # the q0/q1 PSUM banks, so drain those first (q0
                            # frees via its ACT pre-copies)
                            _rope(qtb[0][:, SC * c + co + r0:SC * c + co + r0 + 256],
                                  ps6[0][:, co + r0:co + r0 + 256], c, co + r0, 256,
                                  eng="gp")
                            _rope(qtb[1][:, SC * c + co + r0:SC * c + co + r0 + 256],
                                  ps6[1][:, co + r0:co + r0 + 256], c, co + r0, 256,
                                  eng="dve")
                            _rope(ktb[:, SC * c + co + r0:SC * c + co + r0 + 256],
                                  ps6[4][:, co + r0:co + r0 + 256], c, co + r0, 256,
                                  eng="dve")
                        _rope(qtb[2][:, SC * c + co:SC * c + co + w],
                              ps6[2][:, co:co + w], c, co, w, eng="dve")
                        _rope(qtb[3][:, SC * c + co:SC * c + co + w],
                              ps6[3][:, co:co + w], c, co, w, eng="dve")
                        nc.scalar.copy(vtb[:, SC * c + co:SC * c + co + w],
                                       ps6[5][:, co:co + w])
                        for b2 in range(w // 128):
                            bo = 128 * ((SC // 128) * c + (w // 128) * half + b2)
                            nc.scalar.dma_start_transpose(
                                vnat[:, bo:bo + 128],
                                vtb[:, SC * c + co + 128 * b2:SC * c + co + 128 * (b2 + 1)])

            def attn_stage(c):
                # block-sparse attention for chunk c (past K/V only: sliding
                # window); leftover Wo groups of chunk c-1 fill the rope tail.
                with tc.tile_pool(name="scps", bufs=3, space="PSUM") as psc, \
                     tc.tile_pool(name="pvps", bufs=2, space="PSUM") as ppv, \
                     tc.tile_pool(name="denps", bufs=1, space="PSUM") as pdn:
                    kbs = list(range(max(0, 4 * c - 8), 4 * c + 4))
                    first_kb, last_kb = kbs[0], kbs[-1]
                    while emit_wo_group():
                        pass
                    atbs = []
                    for h0 in range(0, NQH, 2):
                        # process a PAIR of heads per key-block sweep: two
                        # independent score/exp chains per step keep ACT fed.
                        pvs = [ppv.tile([128, 512], F32, tag="pv", name="pv")
                               for _ in range(2)]
                        # one PSUM bank holds both heads' denominator rows
                        # (matmul out base partition must be 0/32/64)
                        pdnt = pdn.tile([33, 512], F32, tag="den", name="den")
                        dens = [pdnt[0:1, :], pdnt[32:33, :]]

                        def emit_pv(kb, pts):
                            # accumulate P@V and row-sums over exact visible slices.
                            sl = kb - 4 * c + 8
                            lo, hi = _slot_region(sl)
                            vsl = vnat[:, 128 * kb:128 * (kb + 1)]
                            for i in range(2):
                                nc.tensor.matmul(pvs[i][:, lo:hi], vsl, pts[i][:, lo:hi],
                                                 start=(kb == first_kb), stop=(kb == last_kb),
                                                 skip_group_check=True)
                                nc.tensor.matmul(dens[i][:, lo:hi], ones_t[:], pts[i][:, lo:hi],
                                                 start=(kb == first_kb), stop=(kb == last_kb),
                                                 skip_group_check=True)

                        pending = []
                        for kb in kbs:
                            sl = kb - 4 * c + 8
                            lo, hi = _slot_region(sl)
                            pts = []
                            for i in range(2):
                                sc = psc.tile([128, 512], F32, tag="sc", name="sc")
                                nc.tensor.matmul(sc[:, lo:hi], ktb[:, 128 * kb:128 * (kb + 1)],
                                                 qtb[h0 + i][:, SC * c + lo:SC * c + hi],
                                                 start=True, stop=True)
                                pt = ppt.tile([128, 512], BF16, tag="pt", name="pt")
                                nc.scalar.activation(pt[:, lo:hi], sc[:, lo:hi], EXP, scale=SCALE)
                                if sl <= 3:
                                    mofs = 128 * sl
                                    nc.vector.tensor_tensor(pt[:, mofs:mofs + 128],
                                                            pt[:, mofs:mofs + 128], mw_t[:], MULT)
                                elif sl >= 8:
                                    mofs = 128 * (sl - 8)
                                    nc.vector.tensor_tensor(pt[:, mofs:mofs + 128],
                                                            pt[:, mofs:mofs + 128], mc_t[:], MULT)
                                pts.append(pt)
                            pending.append((kb, pts))
                            if len(pending) > 2:
                                emit_pv(*pending.pop(0))
                        for pv_item in pending:
                            emit_pv(*pv_item)
                        for i in range(2):
                            # stage P@V out of PSUM via ACT so the bank frees
                            # immediately; normalize from SBUF off the
                            # critical path
                            pvu = pat.tile([128, 512], BF16, tag="pvu",
                                           name="pvu", bufs=2)
                            nc.scalar.copy(pvu[:], pvs[i][:])
                            dre = pmisc.tile([1, 512], BF16, tag="denr", name="denr")
                            with nc.allow_low_precision(reason="softmax denom to bf16"):
                                nc.vector.reciprocal(dre[:], dens[i])
                            dbc = pmisc.tile([128, 512], BF16, tag="denb", name="denb")
                            nc.gpsimd.partition_broadcast(dbc[:], dre[:])
                            at = pat.tile([128, 512], BF16, tag="atb", name="atb")
                            nc.vector.tensor_tensor(at[:], pvu[:], dbc[:], MULT)
                            atbs.append(at)
                return atbs

            for _it in range(niter):
                for c in range(NCH):
                    proj_stage(c)
                    atbs = attn_stage(c)
                    wo_state.update(c=c, atbs=atbs, m=0, obig=None)
            # the last chunk's Wo groups of each iteration ride the next
            # iteration's proj(0)/attn(0); only the final one drains here
            while emit_wo_group():
                pass


_NC_CACHE = {}


def _build(niter=1, fused=True):
    import os
    fused = os.environ.get("KERNEL_FUSED", "1" if fused else "0") == "1"
    key = (niter, fused)
    if key in _NC_CACHE:
        return _NC_CACHE[key]
    nc = bacc.Bacc("TRN2", target_bir_lowering=False, debug=False,
                   enable_asserts=True, num_devices=NCORES)
    dr = {}

    def din(name, shape, dt=F32):
        dr[name] = nc.dram_tensor(name, shape, dt, kind="ExternalInput").ap()

    din("ht", [HID, S], BF16)
    din("wqt", [HID, NQH * D], BF16)
    din("wkt", [HID, D], BF16)
    din("wvt", [HID, D], BF16)
    din("wot", [NQH * D, HID], BF16)
    din("ctab", [128, S])
    din("stab", [128, S])
    din("mcaus", [128, 128], BF16)
    din("mwin", [128, 128], BF16)
    out = nc.dram_tensor("out", [S, HID], BF16, kind="ExternalOutput").ap()

    with tile.TileContext(nc) as tc:
        _program(tc, dr, out, niter, fused)
    nc.compile()
    _NC_CACHE[key] = nc
    return nc


def make_in_maps(inputs):
    hs = np.asarray(inputs["hidden_states"], dtype=np.float32)
    Wq = np.asarray(inputs["Wq"], dtype=np.float32)
    Wk = np.asarray(inputs["Wk"], dtype=np.float32)
    Wv = np.asarray(inputs["Wv"], dtype=np.float32)
    Wo = np.asarray(inputs["Wo"], dtype=np.float32)
    pos = np.asarray(inputs["position_ids"]).reshape(-1)

    assert hs.shape == (1, S, HID), hs.shape
    H = hs[0]
    HT = np.ascontiguousarray(H.T)

    # RoPE tables in [d%64, s] layout (fp32, mirroring the reference math)
    inv = (1.0 / (ROPE_BASE ** (np.arange(0, D, 2, dtype=np.float32) / D))).astype(np.float32)
    ang = pos.astype(np.float32)[None, :] * inv[:, None]          # [64, S]
    cos64 = np.cos(ang).astype(np.float32)
    sin64 = np.sin(ang).astype(np.float32)
    ctab = np.concatenate([cos64, cos64], axis=0)                 # [128, S]
    stab = np.concatenate([-sin64, sin64], axis=0)                # sign-baked

    kk = np.arange(128)[:, None]
    qq = np.arange(128)[None, :]
    mcaus = (qq >= kk).astype(ml_dtypes.bfloat16)   # causal diag block, [k,q]
    mwin = (qq < kk).astype(ml_dtypes.bfloat16)     # window-edge block, [k,q]

    BF = ml_dtypes.bfloat16
    HTB = HT.astype(BF)
    in_maps = []
    for i in range(NCORES):
        in_maps.append({
            "ht": HTB,
            "wqt": np.ascontiguousarray(Wq[512 * i:512 * (i + 1), :].T).astype(BF),
            "wkt": np.ascontiguousarray(Wk[128 * i:128 * (i + 1), :].T).astype(BF),
            "wvt": np.ascontiguousarray(Wv[128 * i:128 * (i + 1), :].T).astype(BF),
            "wot": np.ascontiguousarray(Wo[:, 512 * i:512 * (i + 1)].T).astype(BF),
            "ctab": ctab,
            "stab": stab,
            "mcaus": mcaus,
            "mwin": mwin,
        })

    return in_maps


def kernel(**inputs):
    in_maps = make_in_maps(inputs)
    nc = _build()
    res = run_bass_kernel_spmd(nc, in_maps, core_ids=list(range(NCORES)))

    acc = np.zeros((S, HID), dtype=np.float32)
    for r in res.results:
        acc += r["out"].astype(np.float32)
    return acc.reshape(1, S, HID)



# revision 11
# speedup vs baseline: 1.7454x; 1.7454x over previous
"""Trainium2 Bass kernel: Mistral-style GQA attention with sliding-window mask.

Problem: hidden [1,2048,4096] -> Wq/Wk/Wv projections (32 q heads, 8 kv heads,
head_dim 128) -> RoPE -> sliding-window (1024) causal attention -> Wo.

Sharding: tensor-parallel over heads on 8 NeuronCores. Core i owns KV head i
and query heads 4i..4i+3 (Wq/Wk/Wv row-sharded, Wo column-sharded). Each core
computes partial_i = attn_heads_i @ Wo_i^T in HBM; host sums the 8 partials
(the TP all-reduce) to produce the full output.

All inputs are pre-cast to bf16 and pre-transposed on the host; each core
writes a bf16 [2048, 4096] partial that the host accumulates in fp32.

On-device per core the work is a single software-pipelined stream per
512-query chunk, woven so PE never waits on a phase boundary:
  F(c-1): KV^T projection sweep for chunk c (2 PSUM banks), woven into the
          previous chunk's second attention head-pair as PE filler.
  B..E:   one Q^T projection sweep per query head (ring of 2 PSUM banks);
          RoPE of head h runs on DVE underneath head h+1's sweep; the
          previous chunk's 32 Wo output-tile groups (4 matmuls each, 2
          persistent PSUM banks, ACT stages them out) weave into sweeps
          h0/h1; attention for heads 0/1 (scores -> exp on ACT -> edge
          masks on DVE -> P@V) weaves into sweeps h2/h3.
  F(c):   attention heads 2/3 woven with chunk c+1's KV sweep.
Softmax denominators: exp tiles are summed into a bf16 accumulator on DVE
(all but the last two key-blocks, which ride the PE directly), then folded
into one persistent PSUM bank with a single ones-matmul per head;
reciprocal on DVE, partition-broadcast on GpSimd, normalize on DVE.
"""

import sys

for _p in ("/opt/trn_rl_repo", "/root/.axon_site/_ro/trn_rl_repo"):
    if _p not in sys.path:
        sys.path.insert(0, _p)

import numpy as np
import ml_dtypes

import concourse.bass as bass  # noqa: F401  (registers engine classes)
import concourse.mybir as mybir
import concourse.tile as tile
from concourse import bacc
from concourse.bass_utils import run_bass_kernel_spmd

S = 2048
HID = 4096
D = 128
NQH = 4          # query heads per core
NCORES = 8
SC = 512         # seq chunk
NCH = S // SC
KT = HID // 128  # contraction tiles
WINDOW = 1024
ROPE_BASE = 10000.0
SCALE = 1.0 / float(np.sqrt(D))

F32 = mybir.dt.float32
BF16 = mybir.dt.bfloat16
MULT = mybir.AluOpType.mult
ADD = mybir.AluOpType.add
EXP = mybir.ActivationFunctionType.Exp

# slot sl = kb - 4c + 8 for key-block kb in chunk c; exp-written region.
def _slot_region(sl):
    lo = 128 * (sl - 8) if sl >= 8 else 0
    hi = 512 if sl >= 3 else 128 * (sl + 1)
    return lo, hi


def _chunk_kbs(c):
    return list(range(max(0, 4 * c - 8), 4 * c + 4))


def _program(tc, dr, out, niter=1, fused=True):
    nc = tc.nc
    ht, wqt, wkt, wvt, wot = dr["ht"], dr["wqt"], dr["wkt"], dr["wvt"], dr["wot"]
    ctab, stab, mcaus, mwin = dr["ctab"], dr["stab"], dr["mcaus"], dr["mwin"]

    from contextlib import ExitStack
    with ExitStack() as ctx:
        pw = ctx.enter_context(tc.tile_pool(name="persist", bufs=1))
        prt = ctx.enter_context(tc.tile_pool(name="ropet", bufs=2))

        wqb = pw.tile([128, KT * 512], BF16, name="wqb")
        wkb = pw.tile([128, KT * 128], BF16, name="wkb")
        wvb = pw.tile([128, KT * 128], BF16, name="wvb")
        wob = pw.tile([128, NQH * HID], BF16, name="wob")
        qtb = [pw.tile([128, S], BF16, name=f"qtb{h}") for h in range(NQH)]
        ktb = pw.tile([128, S], BF16, name="ktb")
        vtb = pw.tile([128, S], BF16, name="vtb")
        vnat = pw.tile([128, S], BF16, name="vnat")
        cs_t = pw.tile([128, S], F32, name="cs_t")
        sn_t = pw.tile([128, S], F32, name="sn_t")
        mc_t = pw.tile([128, 128], BF16, name="mc_t")
        mw_t = pw.tile([128, 128], BF16, name="mw_t")
        ones_t = pw.tile([128, 1], BF16, name="ones_t")

        nc.gpsimd.memset(ones_t[:], 1.0)

        def _rope(dst, p, c):
            """dst[bf16 [128,512] slice] = rope(p [128,512], PSUM), chunk c.

            cs_t is cos duplicated across both halves; sn_t is sign-baked
            sin: rows 0:64 = -sin, rows 64:128 = +sin, so
            out = q*cos + rot(q)*sn with rot a plain half-swap.
            """
            lo, hi = SC * c, SC * (c + 1)
            csl = cs_t[:, lo:hi]
            snl = sn_t[:, lo:hi]
            e = nc.vector
            w = 512
            t1 = prt.tile([64, w], F32, tag="rt1", name="rt1")
            t2 = prt.tile([64, w], F32, tag="rt2", name="rt2")
            e.tensor_tensor(t1[:], p[0:64, :], csl[0:64, :], MULT)
            e.tensor_tensor(t2[:], p[64:128, :], snl[0:64, :], MULT)
            e.tensor_tensor(dst[0:64, :], t1[:], t2[:], ADD)
            t3 = prt.tile([64, w], F32, tag="rt1", name="rt3")
            t4 = prt.tile([64, w], F32, tag="rt2", name="rt4")
            e.tensor_tensor(t3[:], p[64:128, :], csl[64:128, :], MULT)
            e.tensor_tensor(t4[:], p[0:64, :], snl[64:128, :], MULT)
            e.tensor_tensor(dst[64:128, :], t3[:], t4[:], ADD)

        phb = ctx.enter_context(tc.tile_pool(name="htbp", bufs=12))
        ppt = ctx.enter_context(tc.tile_pool(name="ptp", bufs=6))
        pmisc = ctx.enter_context(tc.tile_pool(name="miscb", bufs=2))
        pdac = ctx.enter_context(tc.tile_pool(name="daccb", bufs=2))
        pat = ctx.enter_context(tc.tile_pool(name="atbp", bufs=8))
        posb = ctx.enter_context(tc.tile_pool(name="osbp", bufs=2))
        # persistent PSUM: Wo output ring (2 banks) + denominator rows (1)
        pop = ctx.enter_context(tc.tile_pool(name="outps", bufs=2, space="PSUM"))
        pdn = ctx.enter_context(tc.tile_pool(name="denps", bufs=1, space="PSUM"))
        pdnt = pdn.tile([33, 512], F32, name="pdnt")

        hbt = {}      # (c, g) -> hidden-tile [128, 4*512]
        kvt = {}      # c -> (kpp, vpp) PSUM tiles

        def issue_hb(c, g):
            t = phb.tile([128, 2048], BF16, tag="htb", name="hb", bufs=12)
            nc.sync.dma_start(
                t[:].rearrange("p (k j) -> p k j", j=512),
                ht[512 * g:512 * (g + 1), SC * c:SC * (c + 1)].rearrange(
                    "(k p) j -> p k j", p=128))
            hbt[(c, g)] = t

        def kv_sweep(c, pkv):
            kpp = pkv.tile([128, 512], F32, tag="kpp", name="kpp")
            vpp = pkv.tile([128, 512], F32, tag="vpp", name="vpp")
            kvt[c] = (kpp, vpp)
            for k in range(KT):
                hsl = hbt[(c, k // 4)][:, 512 * (k % 4):512 * (k % 4 + 1)]
                first, last = k == 0, k == KT - 1
                nc.tensor.matmul(kpp[:], wkb[:, 128 * k:128 * (k + 1)], hsl,
                                 start=first, stop=last, skip_group_check=True)
                nc.tensor.matmul(vpp[:], wvb[:, 128 * k:128 * (k + 1)], hsl,
                                 start=first, stop=last, skip_group_check=True)
                yield

        def q_sweep(c, h, qpp):
            for k in range(KT):
                hsl = hbt[(c, k // 4)][:, 512 * (k % 4):512 * (k % 4 + 1)]
                nc.tensor.matmul(qpp[:], wqb[:, 512 * k + 128 * h:512 * k + 128 * (h + 1)],
                                 hsl, start=(k == 0), stop=(k == KT - 1),
                                 skip_group_check=True)
                yield

        wo_state = {"obig": None}

        def wo_groups(c, atbs, lo_m, hi_m):
            # one group = one [128q, 512hid] output tile of chunk c
            for m in range(lo_m, hi_m):
                wj, wn = m // 8, m % 8
                if wn % 2 == 0:
                    wo_state["obig"] = posb.tile([128, 1024], BF16,
                                                 tag="osb", name="osb")
                obig = wo_state["obig"]
                po = pop.tile([128, 512], F32, tag="po", name="po")
                for h in range(NQH):
                    nc.tensor.matmul(po[:], atbs[h][:, 128 * wj:128 * (wj + 1)],
                                     wob[:, HID * h + 512 * wn:HID * h + 512 * (wn + 1)],
                                     start=(h == 0), stop=(h == NQH - 1))
                nc.scalar.copy(obig[:, 512 * (wn % 2):512 * (wn % 2 + 1)], po[:])
                if wn % 2 == 1:
                    nc.scalar.dma_start(
                        out[SC * c + 128 * wj:SC * c + 128 * (wj + 1),
                            1024 * (wn // 2):1024 * (wn // 2 + 1)],
                        obig[:])
                yield

        def attn_head(c, h, psc, ppv, atbs_out):
            kbs = _chunk_kbs(c)
            first_kb, last_kb = kbs[0], kbs[-1]
            acc_kbs = kbs[:-2]
            tail_kbs = kbs[-2:]
            dacc = pdac.tile([128, 512], BF16, tag="dac", name="dac", bufs=2)
            nc.vector.memset(dacc[:], 0.0)
            den = pdnt[32 * (h % 2):32 * (h % 2) + 1, :]
            pvt = ppv.tile([128, 512], F32, tag="pv", name="pv")

            def emit_pv(kb, pt):
                lo, hi = _slot_region(kb - 4 * c + 8)
                nc.tensor.matmul(pvt[:, lo:hi], vnat[:, 128 * kb:128 * (kb + 1)],
                                 pt[:, lo:hi], start=(kb == first_kb),
                                 stop=(kb == last_kb), skip_group_check=True)
                if kb in tail_kbs:
                    nc.tensor.matmul(den[:, lo:hi], ones_t[:], pt[:, lo:hi],
                                     start=False, stop=(kb == last_kb),
                                     skip_group_check=True)

            pending = []
            for kb in kbs:
                sl = kb - 4 * c + 8
                lo, hi = _slot_region(sl)
                sct = psc.tile([128, 512], F32, tag="sc", name="sc", bufs=2)
                nc.tensor.matmul(sct[:, lo:hi], ktb[:, 128 * kb:128 * (kb + 1)],
                                 qtb[h][:, SC * c + lo:SC * c + hi],
                                 start=True, stop=True)
                pt = ppt.tile([128, 512], BF16, tag="pt", name="pt", bufs=6)
                nc.scalar.activation(pt[:, lo:hi], sct[:, lo:hi], EXP, scale=SCALE)
                if sl <= 3:
                    mofs = 128 * sl
                    nc.vector.tensor_tensor(pt[:, mofs:mofs + 128],
                                            pt[:, mofs:mofs + 128], mw_t[:], MULT)
                elif sl >= 8:
                    mofs = 128 * (sl - 8)
                    nc.vector.tensor_tensor(pt[:, mofs:mofs + 128],
                                            pt[:, mofs:mofs + 128], mc_t[:], MULT)
                if kb in acc_kbs:
                    nc.vector.tensor_tensor(dacc[:, lo:hi], dacc[:, lo:hi],
                                            pt[:, lo:hi], ADD)
                pending.append((kb, pt))
                if len(pending) > 2:
                    emit_pv(*pending.pop(0))
                yield
            # fold the accumulated exp sums into the denominator row; the two
            # tail key-blocks accumulate directly in emit_pv
            nc.tensor.matmul(den, ones_t[:], dacc[:], start=True, stop=False,
                             skip_group_check=True)
            for item in pending:
                emit_pv(*item)
                yield
            # drain: stage P@V out via ACT so the bank frees, normalize off
            # the critical path
            pvu = pat.tile([128, 512], BF16, tag="pvu", name="pvu", bufs=2)
            nc.scalar.copy(pvu[:], pvt[:])
            dre = pmisc.tile([1, 512], BF16, tag="denr", name="denr")
            with nc.allow_low_precision(reason="softmax denom to bf16"):
                nc.vector.reciprocal(dre[:], den)
            dbc = pmisc.tile([128, 512], BF16, tag="denb", name="denb")
            nc.gpsimd.partition_broadcast(dbc[:], dre[:])
            at = pat.tile([128, 512], BF16, tag="atb", name="atb")
            nc.vector.tensor_tensor(at[:], pvu[:], dbc[:], MULT)
            atbs_out[h] = at

        def chain(*gens):
            for g in gens:
                yield from g

        def drive(*gens_weights):
            gens = list(gens_weights)
            while gens:
                keep = []
                for g, wgt in gens:
                    alive = True
                    for _ in range(wgt):
                        try:
                            next(g)
                        except StopIteration:
                            alive = False
                            break
                    if alive:
                        keep.append((g, wgt))
                gens = keep

        def empty_gen():
            return iter(())

        # ---- initial DMA stream (weights + chunk-0 hidden tiles) ----
        def wdma(dst, src, g, blk):
            nc.sync.dma_start(
                dst[:, blk * g:blk * (g + 1)].rearrange("p (k j) -> p k j", j=blk // 8),
                src[1024 * g:1024 * (g + 1), :].rearrange("(k p) j -> p k j", p=128))

        for g in range(4):
            wdma(wkb, wkt, g, 1024)
            wdma(wvb, wvt, g, 1024)
            issue_hb(0, g)
            wdma(wqb, wqt, g, 4096)
        nc.sync.dma_start(cs_t[:], ctab[:])
        nc.sync.dma_start(sn_t[:], stab[:])
        nc.sync.dma_start(mc_t[:], mcaus[:])
        nc.sync.dma_start(mw_t[:], mwin[:])
        for g in range(4, 8):
            issue_hb(0, g)
        for j in range(8):
            nc.sync.dma_start(
                wob[:, 2048 * j:2048 * (j + 1)],
                wot[128 * (j // 2):128 * (j // 2 + 1),
                    2048 * (j % 2):2048 * (j % 2 + 1)])

        # ---- bootstrap: KV sweep of chunk 0 ----
        pkv_pools = {}
        pkv_pools[0] = tc.alloc_tile_pool(name="kvps0", bufs=1, space="PSUM")
        drive((kv_sweep(0, pkv_pools[0]), 1))

        atbs_prev = None     # previous chunk's normalized attention tiles
        prev_c = None
        seq = [(it, c) for it in range(niter) for c in range(NCH)]
        for idx, (it, c) in enumerate(seq):
            nxt = seq[idx + 1] if idx + 1 < len(seq) else None
            kpp, vpp = kvt.pop(c)

            # V^T -> SBUF, DMA-transpose to natural; rope K. Both read the KV
            # PSUM banks, which then free for this chunk's attention pools.
            nc.scalar.copy(vtb[:, SC * c:SC * (c + 1)], vpp[:])
            for b2 in range(4):
                bo = 128 * (4 * c + b2)
                nc.scalar.dma_start_transpose(
                    vnat[:, bo:bo + 128],
                    vtb[:, SC * c + 128 * b2:SC * c + 128 * (b2 + 1)])
            _rope(ktb[:, SC * c:SC * (c + 1)], kpp[:], c)
            pkv_pools.pop(c).release()

            pq = tc.alloc_tile_pool(name=f"qps{idx}", bufs=2, space="PSUM")
            wo_iter = (wo_groups(prev_c, atbs_prev, 0, 32) if atbs_prev is not None
                       else empty_gen())
            atbs_new = [None] * NQH
            psc = ppv = None
            a01 = None
            for h in range(NQH):
                if nxt is not None:
                    issue_hb(nxt[1], 2 * h)
                    issue_hb(nxt[1], 2 * h + 1)
                qpp = pq.tile([128, 512], F32, tag="qp", name="qp", bufs=2)
                if h == 2:
                    # open attention pools now that the KV banks are free;
                    # weave heads 0/1 attention into the h2/h3 Q sweeps
                    psc = tc.alloc_tile_pool(name=f"scps{idx}", bufs=2, space="PSUM", side="right")
                    ppv = tc.alloc_tile_pool(name=f"pvps{idx}", bufs=1, space="PSUM", side="right")
                    a01 = chain(attn_head(c, 0, psc, ppv, atbs_new),
                                attn_head(c, 1, psc, ppv, atbs_new))
                fill = wo_iter if h < 2 else a01
                drive((q_sweep(c, h, qpp), 2), (fill, 1))
                _rope(qtb[h][:, SC * c:SC * (c + 1)], qpp[:], c)
            pq.release()

            # F: finish heads 0/1, run heads 2/3, weave next chunk's KV sweep
            a23 = chain(attn_head(c, 2, psc, ppv, atbs_new),
                        attn_head(c, 3, psc, ppv, atbs_new))
            if nxt is not None:
                pkv_pools[nxt[1]] = tc.alloc_tile_pool(
                    name=f"kvps{idx + 1}", bufs=1, space="PSUM")
                kv_fill = kv_sweep(nxt[1], pkv_pools[nxt[1]])
            else:
                kv_fill = empty_gen()
            drive((a01, 1), (wo_iter, 1), (a23, 1), (kv_fill, 1))
            ppv.release()
            psc.release()

            atbs_prev = atbs_new
            prev_c = c

        # drain the last chunk's Wo groups
        drive((wo_groups(prev_c, atbs_prev, 0, 32), 1))


_NC_CACHE = {}


def _build(niter=1, fused=True):
    import os
    fused = os.environ.get("KERNEL_FUSED", "1" if fused else "0") == "1"
    key = (niter, fused)
    if key in _NC_CACHE:
        return _NC_CACHE[key]
    nc = bacc.Bacc("TRN2", target_bir_lowering=False, debug=False,
                   enable_asserts=True, num_devices=NCORES)
    dr = {}

    def din(name, shape, dt=F32):
        dr[name] = nc.dram_tensor(name, shape, dt, kind="ExternalInput").ap()

    din("ht", [HID, S], BF16)
    din("wqt", [HID, NQH * D], BF16)
    din("wkt", [HID, D], BF16)
    din("wvt", [HID, D], BF16)
    din("wot", [NQH * D, HID], BF16)
    din("ctab", [128, S])
    din("stab", [128, S])
    din("mcaus", [128, 128], BF16)
    din("mwin", [128, 128], BF16)
    out = nc.dram_tensor("out", [S, HID], BF16, kind="ExternalOutput").ap()

    with tile.TileContext(nc) as tc:
        _program(tc, dr, out, niter, fused)
    nc.compile()
    _NC_CACHE[key] = nc
    return nc


def make_in_maps(inputs):
    hs = np.asarray(inputs["hidden_states"], dtype=np.float32)
    Wq = np.asarray(inputs["Wq"], dtype=np.float32)
    Wk = np.asarray(inputs["Wk"], dtype=np.float32)
    Wv = np.asarray(inputs["Wv"], dtype=np.float32)
    Wo = np.asarray(inputs["Wo"], dtype=np.float32)
    pos = np.asarray(inputs["position_ids"]).reshape(-1)

    assert hs.shape == (1, S, HID), hs.shape
    H = hs[0]
    HT = np.ascontiguousarray(H.T)

    # RoPE tables in [d%64, s] layout (fp32, mirroring the reference math)
    inv = (1.0 / (ROPE_BASE ** (np.arange(0, D, 2, dtype=np.float32) / D))).astype(np.float32)
    ang = pos.astype(np.float32)[None, :] * inv[:, None]          # [64, S]
    cos64 = np.cos(ang).astype(np.float32)
    sin64 = np.sin(ang).astype(np.float32)
    ctab = np.concatenate([cos64, cos64], axis=0)                 # [128, S]
    stab = np.concatenate([-sin64, sin64], axis=0)                # sign-baked

    kk = np.arange(128)[:, None]
    qq = np.arange(128)[None, :]
    mcaus = (qq >= kk).astype(ml_dtypes.bfloat16)   # causal diag block, [k,q]
    mwin = (qq < kk).astype(ml_dtypes.bfloat16)     # window-edge block, [k,q]

    BF = ml_dtypes.bfloat16
    HTB = HT.astype(BF)
    in_maps = []
    for i in range(NCORES):
        in_maps.append({
            "ht": HTB,
            "wqt": np.ascontiguousarray(Wq[512 * i:512 * (i + 1), :].T).astype(BF),
            "wkt": np.ascontiguousarray(Wk[128 * i:128 * (i + 1), :].T).astype(BF),
            "wvt": np.ascontiguousarray(Wv[128 * i:128 * (i + 1), :].T).astype(BF),
            "wot": np.ascontiguousarray(Wo[:, 512 * i:512 * (i + 1)].T).astype(BF),
            "ctab": ctab,
            "stab": stab,
            "mcaus": mcaus,
            "mwin": mwin,
        })

    return in_maps


def kernel(**inputs):
    in_maps = make_in_maps(inputs)
    nc = _build()
    res = run_bass_kernel_spmd(nc, in_maps, core_ids=list(range(NCORES)))

    acc = np.zeros((S, HID), dtype=np.float32)
    for r in res.results:
        acc += r["out"].astype(np.float32)
    return acc.reshape(1, S, HID)
